# revision 9
# baseline (speedup 1.0000x reference)
"""Axial attention kernel for Trainium2 (Bass/Tile), 8-core SPMD.

Problem: q,k,v [2, 8, 16, 32, 32, 64] f32, attention over axis 3 (H=32),
independently for every (b, h, t, w) -> 8192 problems of [L=32, c=64].

Strategy per core (1024 problems = 32 blocks of [H=32, W=32, c=64]):
  - SBUF layout [128, 512]: partition p = j2*32 + l (j2 = w//8), free = (s=w%8)*64 + c.
    DMA moves 2KB-contiguous chunks (full line rate).
  - "pack" for slot s = 4 problems {w = 8*j2 + s}. Pair-transpose q/k on TensorE
    ([128,128] -> c on partitions), then per pack one matmul S^T = K' @ Q'^T
    (K=64, M=128, N=128) with cross-problem blocks masked to zero after exp.
  - P^T = exp(0.125 * S^T) * mask  (no max subtraction: |S/8| <= ~6 for randn data)
  - AV: out = lhsT(P^T).T @ [V'|ones] (K=128, N=65) -> unnormalized out + row sums.
  - normalize on ScalarE with per-partition reciprocal scale; DMA out mirrors input.
"""

import numpy as np

import concourse.bass as bass
import concourse.mybir as mybir
from concourse.tile import TileContext
from concourse.bass_utils import run_bass_kernel_spmd

F32 = mybir.dt.float32


def _legalize_waits(nc):
    """The TPB ISA encodes at most ONE semaphore wait per instruction, and the
    walrus build in this container rejects instructions carrying more. Hoist
    extra waits onto preceding same-engine NoOps (one wait each)."""
    for f in nc.m.functions:
        for bb in list(f.blocks):
            insts = list(bb.instructions)
            offenders = [
                i
                for i in insts
                if i.sync_info is not None and len(i.sync_info.on_wait) > 1
            ]
            if not offenders:
                continue
            plan = {}
            new_names = set()
            for i in offenders:
                waits = list(i.sync_info.on_wait)
                nops = []
                for w in waits[:-1]:
                    nop = nc.engines[i.engine].nop(hint="lgw").ins
                    nop.sync_info = mybir.SyncInfo(on_wait=[w], on_update=[])
                    nops.append(nop)
                    new_names.add(nop.name)
                i.sync_info = mybir.SyncInfo(
                    on_wait=[waits[-1]], on_update=list(i.sync_info.on_update)
                )
                plan[i.name] = nops
            for bb2 in f.blocks:
                lst = list(bb2.instructions)
                if any(x.name in new_names for x in lst):
                    bb2.instructions = [x for x in lst if x.name not in new_names]
            out = []
            for i in bb.instructions:
                if i.name in plan:
                    out.extend(plan[i.name])
                out.append(i)
            bb.instructions = out

N_CORES = 8
NB = 32  # (b*h*t) blocks per core: 2*8*16 / 8
H, W, C = 32, 32, 64
BLK = H * W * C  # 65536 elements per block
SCALE = 1.0 / 8.0  # 1/sqrt(64)


def _build_nc(iters=1):
    nc = bass.Bass()
    q = nc.dram_tensor("q", [NB, H, W, C], F32, kind="ExternalInput")
    k = nc.dram_tensor("k", [NB, H, W, C], F32, kind="ExternalInput")
    v = nc.dram_tensor("v", [NB, H, W, C], F32, kind="ExternalInput")
    mask_c = nc.dram_tensor("mask_c", [128, 512], F32, kind="ExternalInput")
    ident_c = nc.dram_tensor("ident_c", [128, 128], F32, kind="ExternalInput")
    out = nc.dram_tensor("out", [NB, H, W, C], F32, kind="ExternalOutput")

    # [NB, j2, l, (s c)] view: partition dims (j2, l), free 512 contiguous
    qr = q[:].rearrange("n l (j s) c -> n j l (s c)", j=4)
    kr = k[:].rearrange("n l (j s) c -> n j l (s c)", j=4)
    vr = v[:].rearrange("n l (j s) c -> n j l (s c)", j=4)
    outr = out[:].rearrange("n l (j s) c -> n j l (s c)", j=4)

    with TileContext(nc) as tc:
        with (
            tc.tile_pool(name="const", bufs=1) as const,
            tc.tile_pool(name="io", bufs=3) as io,
            tc.tile_pool(name="work", bufs=3) as work,
            tc.tile_pool(name="pstr", bufs=2, space="PSUM") as pstr,
            tc.tile_pool(name="psqk", bufs=2, space="PSUM") as psqk,
            tc.tile_pool(name="psav", bufs=2, space="PSUM") as psav,
        ):
            mask_sb = const.tile([128, 512], F32)
            nc.sync.dma_start(out=mask_sb[:], in_=mask_c[:])
            ident_sb = const.tile([128, 128], F32)
            nc.sync.dma_start(out=ident_sb[:], in_=ident_c[:])

            for n_rep in range(NB * iters):
                n = n_rep % NB
                qt = io.tile([128, 512], F32, tag="qt")
                kt = io.tile([128, 512], F32, tag="kt")
                vt = io.tile([128, 512], F32, tag="vt")
                nc.sync.dma_start(out=qt[:], in_=qr[n])
                nc.sync.dma_start(out=kt[:], in_=kr[n])
                nc.sync.dma_start(out=vt[:], in_=vr[n])

                # v65: 8 slots of [64 v-cols | 1.0] for the fused sum column
                v65 = work.tile([128, 520], F32, tag="v65")
                v65_3d = v65[:].rearrange("p (s x) -> p s x", s=8)
                nc.gpsimd.tensor_copy(
                    out=v65_3d[:, :, 0:64],
                    in_=vt[:].rearrange("p (s x) -> p s x", s=8),
                )
                nc.gpsimd.memset(v65_3d[:, :, 64:65], 1.0)

                # --- transposes: pairs g2 in 0..3, each [128,128] -> (c on partitions)
                # tr tile cols: [0:128] = q-pair(2h), [128:256] = k-pair(2h),
                #               [256:384] = q-pair(2h+1), [384:512] = k-pair(2h+1)
                trs = []
                for h in range(2):
                    tr = pstr.tile([128, 512], F32, tag="tr")
                    for i in range(2):
                        g2 = 2 * h + i
                        nc.tensor.transpose(
                            tr[:, i * 256 : i * 256 + 128],
                            qt[:, g2 * 128 : (g2 + 1) * 128],
                            ident_sb[:],
                        )
                        nc.tensor.transpose(
                            tr[:, i * 256 + 128 : i * 256 + 256],
                            kt[:, g2 * 128 : (g2 + 1) * 128],
                            ident_sb[:],
                        )
                    trs.append(tr)

                qkTs = []
                for h in range(2):
                    qkT = work.tile([128, 512], F32, tag="qkT")
                    nc.vector.tensor_copy(out=qkT[:], in_=trs[h][:])
                    qkTs.append(qkT)

                # --- QK matmuls: S^T per slot.
                # HW constraint: matmuls with different tile_position row
                # offsets must not write the same PSUM bank -> sp=0 slots go
                # to tile A, sp=1 slots to tile B.
                # pT column blocks (per half): pos -> slot [4h, 4h+2, 4h+1, 4h+3]
                sqkAs, sqkBs = [], []
                for h in range(2):
                    sqkA = psqk.tile([128, 256], F32, tag="sqkA")
                    sqkB = psqk.tile([128, 256], F32, tag="sqkB")
                    for idx in range(4):
                        s = 4 * h + idx
                        sp, i = s % 2, (s // 2) % 2
                        qT = qkTs[h][
                            sp * 64 : (sp + 1) * 64, i * 256 : i * 256 + 128
                        ]
                        kT = qkTs[h][
                            sp * 64 : (sp + 1) * 64, i * 256 + 128 : i * 256 + 256
                        ]
                        dst = sqkA if sp == 0 else sqkB
                        nc.tensor.matmul(
                            dst[:, i * 128 : (i + 1) * 128],
                            lhsT=kT,
                            rhs=qT,
                            start=True,
                            stop=True,
                        )
                    sqkAs.append(sqkA)
                    sqkBs.append(sqkB)

                # --- softmax numerator: P^T = exp(scale * S^T) * mask
                pTs = []
                for h in range(2):
                    pT = work.tile([128, 512], F32, tag="pT")
                    nc.scalar.activation(
                        out=pT[:, 0:256],
                        in_=sqkAs[h][:],
                        func=mybir.ActivationFunctionType.Exp,
                        scale=SCALE,
                    )
                    nc.scalar.activation(
                        out=pT[:, 256:512],
                        in_=sqkBs[h][:],
                        func=mybir.ActivationFunctionType.Exp,
                        scale=SCALE,
                    )
                    nc.vector.tensor_mul(pT[:], pT[:], mask_sb[:])
                    pTs.append(pT)

                # --- AV matmuls (+ sum column), reciprocal, normalize
                ot = io.tile([128, 512], F32, tag="ot")
                rcp = work.tile([128, 8], F32, tag="rcp")
                savs = []
                for h in range(2):
                    slot_at_pos = [4 * h, 4 * h + 2, 4 * h + 1, 4 * h + 3]
                    sav = psav.tile([128, 260], F32, tag="sav")
                    for pos in range(4):
                        s = slot_at_pos[pos]
                        nc.tensor.matmul(
                            sav[:, pos * 65 : (pos + 1) * 65],
                            lhsT=pTs[h][:, pos * 128 : (pos + 1) * 128],
                            rhs=v65[:, s * 65 : (s + 1) * 65],
                            start=True,
                            stop=True,
                        )
                    savs.append(sav)

                for h in range(2):
                    sums = savs[h][:].rearrange("p (s x) -> p s x", s=4)[:, :, 64]
                    nc.vector.reciprocal(out=rcp[:, h * 4 : h * 4 + 4], in_=sums)

                for h in range(2):
                    slot_at_pos = [4 * h, 4 * h + 2, 4 * h + 1, 4 * h + 3]
                    for pos in range(4):
                        s = slot_at_pos[pos]
                        nc.scalar.mul(
                            out=ot[:, s * 64 : (s + 1) * 64],
                            in_=savs[h][:, pos * 65 : pos * 65 + 64],
                            mul=rcp[:, h * 4 + pos : h * 4 + pos + 1],
                        )

                nc.sync.dma_start(out=outr[n], in_=ot[:])

    _legalize_waits(nc)
    return nc


_NC = {}


def _get_nc(iters=1):
    if iters not in _NC:
        _NC[iters] = _build_nc(iters)
    return _NC[iters]


def _make_consts():
    mask = np.zeros((128, 512), dtype=np.float32)
    for idx in range(4):
        for j2 in range(4):
            mask[
                j2 * 32 : (j2 + 1) * 32, idx * 128 + j2 * 32 : idx * 128 + (j2 + 1) * 32
            ] = 1.0
    ident = np.eye(128, dtype=np.float32)
    return mask, ident


def _make_in_maps(q, k, v):
    mask, ident = _make_consts()
    qs = np.ascontiguousarray(q, dtype=np.float32).reshape(N_CORES, NB, H, W, C)
    ks = np.ascontiguousarray(k, dtype=np.float32).reshape(N_CORES, NB, H, W, C)
    vs = np.ascontiguousarray(v, dtype=np.float32).reshape(N_CORES, NB, H, W, C)
    return [
        {"q": qs[i], "k": ks[i], "v": vs[i], "mask_c": mask, "ident_c": ident}
        for i in range(N_CORES)
    ]


def run_hw(q, k, v, trace=False, iters=1, **kwargs):
    """Run on hardware; returns (full_output, BassKernelResults)."""
    nc = _get_nc(iters)
    in_maps = _make_in_maps(q, k, v)
    res = run_bass_kernel_spmd(nc, in_maps, list(range(N_CORES)), trace=trace, **kwargs)
    full = np.concatenate(
        [res.results[i]["out"].reshape(1, NB, H, W, C) for i in range(N_CORES)], axis=0
    ).reshape(2, 8, 16, 32, 32, 64)
    return full.astype(np.float32), res


def kernel(q, k, v):
    out, _ = run_hw(q, k, v, trace=False)
    return out


# revision 20
# speedup vs baseline: 236.1651x; 236.1651x over previous
"""Axial attention kernel for Trainium2 (Bass/Tile), 8-core SPMD.

Problem: q,k,v [2, 8, 16, 32, 32, 64] f32, attention over axis 3 (H=32),
independently for every (b, h, t, w) -> 8192 problems of [L=32, c=64].

Strategy per core (1024 problems = 32 blocks of [H=32, W=32, c=64]):
  - SBUF layout [128, 512]: partition p = j2*32 + l (j2 = w//8), free = (s=w%8)*64 + c.
    DMA moves 2KB-contiguous chunks (full line rate); loads are prefetched two
    blocks ahead so stores never stall the HWDGE FIFO.
  - "pack" for slot s = 4 problems {w = 8*j2 + s}. Pair-transpose q/k on TensorE
    ([128,128] -> c on partitions), then per pack one matmul S^T = K' @ Q'^T
    (K=64, M=128, N=128) with cross-problem blocks masked to zero after exp.
    HW constraint found empirically: matmuls with different tile_position row
    offsets must write different PSUM banks, else the device hangs.
  - P^T = exp(0.125 * S^T) * mask  (no max subtraction: |S/8| <= ~6 for randn data)
  - AV: out = lhsT(P^T).T @ [V'|ones] (K=128, N=65) -> unnormalized out + row sums
    in one matmul (softmax denominator needs a partition-dim sum, which only the
    PE can do; the ones column makes it free).
  - normalize on ScalarE with per-partition reciprocal scale; DMA out mirrors input.
  - all-fp32 compute (rel err ~3e-6). A bf16 variant was tried and is *slower* on
    this hardware: non-fp32 matmuls emit separate InstLdweights instructions and
    per-instruction overhead dominates at these tiny matmul sizes.
"""

import numpy as np

import concourse.bass as bass
import concourse.mybir as mybir
from concourse.tile import TileContext
from concourse.bass_utils import run_bass_kernel_spmd

F32 = mybir.dt.float32


def _legalize_waits(nc):
    """The TPB ISA encodes at most ONE semaphore wait per instruction, and the
    walrus build in this container rejects instructions carrying more. Hoist
    extra waits onto preceding same-engine NoOps (one wait each)."""
    for f in nc.m.functions:
        for bb in list(f.blocks):
            insts = list(bb.instructions)
            offenders = [
                i
                for i in insts
                if i.sync_info is not None and len(i.sync_info.on_wait) > 1
            ]
            if not offenders:
                continue
            plan = {}
            new_names = set()
            for i in offenders:
                waits = list(i.sync_info.on_wait)
                nops = []
                for w in waits[:-1]:
                    nop = nc.engines[i.engine].nop(hint="lgw").ins
                    nop.sync_info = mybir.SyncInfo(on_wait=[w], on_update=[])
                    nops.append(nop)
                    new_names.add(nop.name)
                i.sync_info = mybir.SyncInfo(
                    on_wait=[waits[-1]], on_update=list(i.sync_info.on_update)
                )
                plan[i.name] = nops
            for bb2 in f.blocks:
                lst = list(bb2.instructions)
                if any(x.name in new_names for x in lst):
                    bb2.instructions = [x for x in lst if x.name not in new_names]
            out = []
            for i in bb.instructions:
                if i.name in plan:
                    out.extend(plan[i.name])
                out.append(i)
            bb.instructions = out

N_CORES = 8
NB = 32  # (b*h*t) blocks per core: 2*8*16 / 8
H, W, C = 32, 32, 64
BLK = H * W * C  # 65536 elements per block
SCALE = 1.0 / 8.0  # 1/sqrt(64)


def _build_nc(iters=1):
    nc = bass.Bass()
    q = nc.dram_tensor("q", [NB, H, W, C], F32, kind="ExternalInput")
    k = nc.dram_tensor("k", [NB, H, W, C], F32, kind="ExternalInput")
    v = nc.dram_tensor("v", [NB, H, W, C], F32, kind="ExternalInput")
    mask_c = nc.dram_tensor("mask_c", [128, 512], F32, kind="ExternalInput")
    ident_c = nc.dram_tensor("ident_c", [128, 128], F32, kind="ExternalInput")
    out = nc.dram_tensor("out", [NB, H, W, C], F32, kind="ExternalOutput")

    # [NB, j2, l, (s c)] view: partition dims (j2, l), free 512 contiguous
    qr = q[:].rearrange("n l (j s) c -> n j l (s c)", j=4)
    kr = k[:].rearrange("n l (j s) c -> n j l (s c)", j=4)
    vr = v[:].rearrange("n l (j s) c -> n j l (s c)", j=4)
    outr = out[:].rearrange("n l (j s) c -> n j l (s c)", j=4)

    with TileContext(nc) as tc:
        with (
            tc.tile_pool(name="const", bufs=1) as const,
            tc.tile_pool(name="io", bufs=6) as io,
            tc.tile_pool(name="work", bufs=5) as work,
            tc.tile_pool(name="pstr", bufs=2, space="PSUM") as pstr,
            tc.tile_pool(name="psqk", bufs=2, space="PSUM") as psqk,
            tc.tile_pool(name="psav", bufs=2, space="PSUM") as psav,
        ):
            mask_sb = const.tile([128, 512], F32)
            nc.sync.dma_start(out=mask_sb[:], in_=mask_c[:])
            ident_sb = const.tile([128, 128], F32)
            nc.sync.dma_start(out=ident_sb[:], in_=ident_c[:])

            NTOT = NB * iters
            LOOKAHEAD = 2

            def issue_loads(n_rep):
                n = n_rep % NB
                qt = io.tile([128, 512], F32, tag="qt", name=f"qt{n_rep}")
                kt = io.tile([128, 512], F32, tag="kt", name=f"kt{n_rep}")
                vt = io.tile([128, 512], F32, tag="vt", name=f"vt{n_rep}")
                nc.sync.dma_start(out=qt[:], in_=qr[n])
                nc.sync.dma_start(out=kt[:], in_=kr[n])
                nc.sync.dma_start(out=vt[:], in_=vr[n])
                return qt, kt, vt

            pending = {}
            for n_rep in range(min(LOOKAHEAD + 1, NTOT)):
                pending[n_rep] = issue_loads(n_rep)

            for n_rep in range(NTOT):
                n = n_rep % NB
                qt, kt, vt = pending.pop(n_rep)
                if n_rep + LOOKAHEAD + 1 < NTOT:
                    pending[n_rep + LOOKAHEAD + 1] = issue_loads(n_rep + LOOKAHEAD + 1)

                # v65: 8 slots of [64 v-cols | 1.0] for the fused sum column
                v65 = work.tile([128, 520], F32, tag="v65")
                v65_3d = v65[:].rearrange("p (s x) -> p s x", s=8)
                nc.gpsimd.tensor_copy(
                    out=v65_3d[:, :, 0:64],
                    in_=vt[:].rearrange("p (s x) -> p s x", s=8),
                )
                nc.gpsimd.memset(v65_3d[:, :, 64:65], 1.0)

                # --- transposes: pairs g2 in 0..3, each [128,128] -> (c on partitions)
                # tr tile cols: [0:128] = q-pair(2h), [128:256] = k-pair(2h),
                #               [256:384] = q-pair(2h+1), [384:512] = k-pair(2h+1)
                trs = []
                for h in range(2):
                    tr = pstr.tile([128, 512], F32, tag="tr")
                    for i in range(2):
                        g2 = 2 * h + i
                        nc.tensor.transpose(
                            tr[:, i * 256 : i * 256 + 128],
                            qt[:, g2 * 128 : (g2 + 1) * 128],
                            ident_sb[:],
                        )
                        nc.tensor.transpose(
                            tr[:, i * 256 + 128 : i * 256 + 256],
                            kt[:, g2 * 128 : (g2 + 1) * 128],
                            ident_sb[:],
                        )
                    trs.append(tr)

                qkTs = []
                for h in range(2):
                    qkT = work.tile([128, 512], F32, tag="qkT")
                    nc.vector.tensor_copy(out=qkT[:], in_=trs[h][:])
                    qkTs.append(qkT)

                # --- QK matmuls: S^T per slot.
                # HW constraint: matmuls with different tile_position row
                # offsets must not write the same PSUM bank -> sp=0 slots go
                # to tile A, sp=1 slots to tile B.
                # pT column blocks (per half): pos -> slot [4h, 4h+2, 4h+1, 4h+3]
                sqkAs, sqkBs = [], []
                for h in range(2):
                    sqkA = psqk.tile([128, 256], F32, tag="sqkA")
                    sqkB = psqk.tile([128, 256], F32, tag="sqkB")
                    for idx in range(4):
                        s = 4 * h + idx
                        sp, i = s % 2, (s // 2) % 2
                        qT = qkTs[h][
                            sp * 64 : (sp + 1) * 64, i * 256 : i * 256 + 128
                        ]
                        kT = qkTs[h][
                            sp * 64 : (sp + 1) * 64, i * 256 + 128 : i * 256 + 256
                        ]
                        dst = sqkA if sp == 0 else sqkB
                        nc.tensor.matmul(
                            dst[:, i * 128 : (i + 1) * 128],
                            lhsT=kT,
                            rhs=qT,
                            start=True,
                            stop=True,
                        )
                    sqkAs.append(sqkA)
                    sqkBs.append(sqkB)

                # --- softmax numerator: P^T = exp(scale * S^T) * mask
                pTs = []
                for h in range(2):
                    pT = work.tile([128, 512], F32, tag="pT")
                    nc.scalar.activation(
                        out=pT[:, 0:256],
                        in_=sqkAs[h][:],
                        func=mybir.ActivationFunctionType.Exp,
                        scale=SCALE,
                    )
                    nc.scalar.activation(
                        out=pT[:, 256:512],
                        in_=sqkBs[h][:],
                        func=mybir.ActivationFunctionType.Exp,
                        scale=SCALE,
                    )
                    nc.vector.tensor_mul(pT[:], pT[:], mask_sb[:])
                    pTs.append(pT)

                # --- AV matmuls (+ sum column), reciprocal, normalize
                ot = io.tile([128, 512], F32, tag="ot")
                rcp = work.tile([128, 8], F32, tag="rcp")
                savs = []
                for h in range(2):
                    slot_at_pos = [4 * h, 4 * h + 2, 4 * h + 1, 4 * h + 3]
                    sav = psav.tile([128, 260], F32, tag="sav")
                    for pos in range(4):
                        s = slot_at_pos[pos]
                        nc.tensor.matmul(
                            sav[:, pos * 65 : (pos + 1) * 65],
                            lhsT=pTs[h][:, pos * 128 : (pos + 1) * 128],
                            rhs=v65[:, s * 65 : (s + 1) * 65],
                            start=True,
                            stop=True,
                        )
                    savs.append(sav)

                for h in range(2):
                    sums = savs[h][:].rearrange("p (s x) -> p s x", s=4)[:, :, 64]
                    nc.vector.reciprocal(out=rcp[:, h * 4 : h * 4 + 4], in_=sums)

                for h in range(2):
                    slot_at_pos = [4 * h, 4 * h + 2, 4 * h + 1, 4 * h + 3]
                    for pos in range(4):
                        s = slot_at_pos[pos]
                        nc.scalar.mul(
                            out=ot[:, s * 64 : (s + 1) * 64],
                            in_=savs[h][:, pos * 65 : pos * 65 + 64],
                            mul=rcp[:, h * 4 + pos : h * 4 + pos + 1],
                        )

                nc.sync.dma_start(out=outr[n], in_=ot[:])

    _legalize_waits(nc)
    return nc


_NC = {}


def _get_nc(iters=1):
    if iters not in _NC:
        _NC[iters] = _build_nc(iters)
    return _NC[iters]


def _make_consts():
    mask = np.zeros((128, 512), dtype=np.float32)
    for idx in range(4):
        for j2 in range(4):
            mask[
                j2 * 32 : (j2 + 1) * 32, idx * 128 + j2 * 32 : idx * 128 + (j2 + 1) * 32
            ] = 1.0
    ident = np.eye(128, dtype=np.float32)
    return mask, ident


def _make_in_maps(q, k, v):
    mask, ident = _make_consts()
    qs = np.ascontiguousarray(q, dtype=np.float32).reshape(N_CORES, NB, H, W, C)
    ks = np.ascontiguousarray(k, dtype=np.float32).reshape(N_CORES, NB, H, W, C)
    vs = np.ascontiguousarray(v, dtype=np.float32).reshape(N_CORES, NB, H, W, C)
    return [
        {"q": qs[i], "k": ks[i], "v": vs[i], "mask_c": mask, "ident_c": ident}
        for i in range(N_CORES)
    ]


def run_hw(q, k, v, trace=False, iters=1, **kwargs):
    """Run on hardware; returns (full_output, BassKernelResults)."""
    nc = _get_nc(iters)
    in_maps = _make_in_maps(q, k, v)
    res = run_bass_kernel_spmd(nc, in_maps, list(range(N_CORES)), trace=trace, **kwargs)
    full = np.concatenate(
        [res.results[i]["out"].reshape(1, NB, H, W, C) for i in range(N_CORES)], axis=0
    ).reshape(2, 8, 16, 32, 32, 64)
    return full.astype(np.float32), res


def kernel(q, k, v):
    out, _ = run_hw(q, k, v, trace=False)
    return out
